# revision 2
# baseline (speedup 1.0000x reference)
"""CAM (channel attention) module kernel for Trainium2, SPMD over 8 NeuronCores.

Reference computation (per batch b):
    V = x[b].reshape(C, N)                    # C=512, N=4096
    E = V @ V.T                               # C x C
    A = softmax(max_row(E) - E, axis=-1)      # == exp(min_row(E) - E) / rowsum
    out[b] = gamma * (A @ V) + x[b]

Sharding: data-parallel over batch. B=16 -> 2 batches per core.
Compute in bf16 (fp32 accumulate in PSUM), residual added in bf16,
output fp32.
"""

import numpy as np
from contextlib import ExitStack

import ml_dtypes

import concourse.bass as bass
import concourse.tile as tile
from concourse import bacc, mybir
from concourse.bass_utils import run_bass_kernel_spmd

B, C, HH, WW = 16, 512, 64, 64
N = HH * WW              # 4096
NCORES = 8
BPC = B // NCORES        # batches per core = 2

CT = C // 128            # 4 c-tiles
NK = N // 128            # 32 n-blocks (contraction chunks for E)
NB = N // 512            # 8 n-chunks of 512 for the output matmul

FP32 = mybir.dt.float32
BF16 = mybir.dt.bfloat16


def _build_kernel():
    nc = bacc.Bacc(
        "TRN2",
        target_bir_lowering=False,
        debug=False,
        num_devices=NCORES,
    )

    x_ext = nc.dram_tensor("x", [BPC, C, N], FP32, kind="ExternalInput")
    g_ext = nc.dram_tensor("gamma", [1, 1], FP32, kind="ExternalInput")
    id_ext = nc.dram_tensor("ident", [128, 128], BF16, kind="ExternalInput")
    out_ext = nc.dram_tensor("out", [BPC, C, N], FP32, kind="ExternalOutput")

    with tile.TileContext(nc) as tc:
        with ExitStack() as ctx:
            _body(ctx, tc, nc, x_ext, g_ext, id_ext, out_ext)

    nc.compile()
    return nc


def _body(ctx, tc, nc, x_ext, g_ext, id_ext, out_ext):
    consts = ctx.enter_context(tc.tile_pool(name="consts", bufs=1))
    xin_pool = ctx.enter_context(tc.tile_pool(name="xin", bufs=4))
    vn_pool = ctx.enter_context(tc.tile_pool(name="vn", bufs=2 * CT))
    vt_pool = ctx.enter_context(tc.tile_pool(name="vt", bufs=NK))
    tx_pool = ctx.enter_context(tc.tile_pool(name="tx", bufs=CT))
    at_pool = ctx.enter_context(tc.tile_pool(name="at", bufs=CT))
    st_pool = ctx.enter_context(tc.tile_pool(name="st", bufs=2 * CT))
    out_pool = ctx.enter_context(tc.tile_pool(name="osb", bufs=4))

    ps_tr = ctx.enter_context(tc.tile_pool(name="ps_tr", bufs=2, space="PSUM"))
    ps_e = ctx.enter_context(tc.tile_pool(name="ps_e", bufs=2, space="PSUM"))
    ps_u = ctx.enter_context(tc.tile_pool(name="ps_u", bufs=2, space="PSUM"))

    ident = consts.tile([128, 128], BF16, name="ident")
    nc.sync.dma_start(ident[:], id_ext[:, :])

    gam = consts.tile([1, 1], FP32, name="gam")
    nc.sync.dma_start(gam[:], g_ext[:, :])
    gbc = consts.tile([128, 1], FP32, name="gbc")
    nc.gpsimd.partition_broadcast(gbc[:], gam[:], channels=128)

    for b in range(BPC):
        # ---- load + convert to bf16 (natural layout: c on partitions) ----
        vn = []
        for ct in range(CT):
            xin = xin_pool.tile([128, N], FP32, name="xin", tag="xin")
            nc.sync.dma_start(xin[:], x_ext[b, ct * 128:(ct + 1) * 128, :])
            v = vn_pool.tile([128, N], BF16, name="vn", tag="vn")
            nc.scalar.copy(v[:], xin[:])
            vn.append(v)

        # ---- transpose V -> Vt via identity matmuls (out = V_blk.T @ I) ----
        vt = []
        for nb in range(NK):
            ps = ps_tr.tile([128, 512], FP32, name="ps_tr", tag="ps_tr")
            for ct in range(CT):
                nc.tensor.matmul(
                    ps[:, ct * 128:(ct + 1) * 128],
                    lhsT=vn[ct][:, nb * 128:(nb + 1) * 128],
                    rhs=ident[:],
                    start=True,
                    stop=True,
                )
            t = vt_pool.tile([128, 512], BF16, name="vt", tag="vt")
            if nb % 2 == 0:
                nc.scalar.copy(t[:], ps[:])
            else:
                nc.vector.tensor_copy(t[:], ps[:])
            vt.append(t)

        # ---- E = V @ V.T  (per c-tile row of E), then exp(min - E) ----
        tx = []
        rsg = []
        for ct in range(CT):
            pse = ps_e.tile([128, 512], FP32, name="ps_e", tag="ps_e")
            for k in range(NK):
                nc.tensor.matmul(
                    pse[:],
                    lhsT=vt[k][:, ct * 128:(ct + 1) * 128],
                    rhs=vt[k][:],
                    start=(k == 0),
                    stop=(k == NK - 1),
                )
            mmin = st_pool.tile([128, 1], FP32, name="mmin", tag="mmin")
            nc.vector.tensor_reduce(
                out=mmin[:], in_=pse[:],
                axis=mybir.AxisListType.X, op=mybir.AluOpType.min,
            )
            t = tx_pool.tile([128, 512], BF16, name="tx", tag="tx")
            ssum = st_pool.tile([128, 1], FP32, name="ssum", tag="ssum")
            # t = exp(min_row(E) - E), ssum = rowsum(t)
            nc.scalar.activation(
                t[:], pse[:], mybir.ActivationFunctionType.Exp,
                bias=mmin[:], scale=-1.0, accum_out=ssum[:],
            )
            rs = st_pool.tile([128, 1], FP32, name="rs", tag="rs")
            nc.vector.reciprocal(rs[:], ssum[:])
            rg = st_pool.tile([128, 1], FP32, name="rg", tag="rg")
            nc.vector.tensor_mul(rg[:], rs[:], gbc[:])   # gamma / S_c
            tx.append(t)
            rsg.append(rg)

        # ---- A^T (unnormalized) via identity matmuls ----
        at = []
        for dj in range(CT):
            ps = ps_tr.tile([128, 512], FP32, name="ps_at", tag="ps_tr")
            for ct in range(CT):
                nc.tensor.matmul(
                    ps[:, ct * 128:(ct + 1) * 128],
                    lhsT=tx[ct][:, dj * 128:(dj + 1) * 128],
                    rhs=ident[:],
                    start=True,
                    stop=True,
                )
            a = at_pool.tile([128, 512], BF16, name="at", tag="at")
            nc.vector.tensor_copy(a[:], ps[:])
            at.append(a)

        # ---- U = T @ V ; out = (gamma/S_c) * U + x ----
        for ct in range(CT):
            for nb in range(NB):
                psu = ps_u.tile([128, 512], FP32, name="ps_u", tag="ps_u")
                for dj in range(CT):
                    nc.tensor.matmul(
                        psu[:],
                        lhsT=at[dj][:, ct * 128:(ct + 1) * 128],
                        rhs=vn[dj][:, nb * 512:(nb + 1) * 512],
                        start=(dj == 0),
                        stop=(dj == CT - 1),
                    )
                o = out_pool.tile([128, 512], FP32, name="osb", tag="osb")
                nc.vector.scalar_tensor_tensor(
                    out=o[:],
                    in0=psu[:],
                    scalar=rsg[ct][:],
                    in1=vn[ct][:, nb * 512:(nb + 1) * 512],
                    op0=mybir.AluOpType.mult,
                    op1=mybir.AluOpType.add,
                )
                nc.gpsimd.dma_start(
                    out_ext[b, ct * 128:(ct + 1) * 128, nb * 512:(nb + 1) * 512],
                    o[:],
                )


_NC_CACHE = {}


def _get_nc():
    if "nc" not in _NC_CACHE:
        _NC_CACHE["nc"] = _build_kernel()
    return _NC_CACHE["nc"]


def kernel(x: np.ndarray, gamma: np.ndarray) -> np.ndarray:
    assert x.shape == (B, C, HH, WW), x.shape
    nc = _get_nc()

    xr = np.ascontiguousarray(x, dtype=np.float32).reshape(B, C, N)
    g2 = np.asarray(gamma, dtype=np.float32).reshape(1, 1)
    ident = np.eye(128, dtype=ml_dtypes.bfloat16)

    in_maps = []
    for i in range(NCORES):
        in_maps.append({
            "x": xr[i * BPC:(i + 1) * BPC],
            "gamma": g2,
            "ident": ident,
        })

    res = run_bass_kernel_spmd(nc, in_maps, core_ids=list(range(NCORES)))
    outs = [res.results[i]["out"] for i in range(NCORES)]
    full = np.concatenate(outs, axis=0).reshape(B, C, HH, WW)
    return full.astype(np.float32)


# revision 5
# speedup vs baseline: 10.6805x; 10.6805x over previous
"""CAM (channel attention) module kernel for Trainium2, SPMD over 8 NeuronCores.

Reference computation (per batch b):
    V = x[b].reshape(C, N)                    # C=512, N=4096
    E = V @ V.T                               # C x C
    A = softmax(max_row(E) - E, axis=-1)      # == exp(min_row(E) - E) / rowsum
    out[b] = gamma * (A @ V) + x[b]

Sharding: data-parallel over batch. B=16 -> 2 batches per core.
Compute in bf16 (fp32 accumulate in PSUM), residual added in bf16,
output fp32.
"""

import numpy as np
from contextlib import ExitStack

import ml_dtypes

import concourse.bass as bass
import concourse.tile as tile
from concourse import bacc, mybir
from concourse.bass_utils import run_bass_kernel_spmd

B, C, HH, WW = 16, 512, 64, 64
N = HH * WW              # 4096
NCORES = 8
BPC = B // NCORES        # batches per core = 2

CT = C // 128            # 4 c-tiles
NK = N // 128            # 32 n-blocks (contraction chunks for E)
NB = N // 512            # 8 n-chunks of 512 for the output matmul

FP32 = mybir.dt.float32
BF16 = mybir.dt.bfloat16


def _build_kernel(reps=1):
    nc = bacc.Bacc(
        "TRN2",
        target_bir_lowering=False,
        debug=False,
        num_devices=NCORES,
    )

    x_ext = nc.dram_tensor("x", [BPC, C, N], FP32, kind="ExternalInput")
    g_ext = nc.dram_tensor("gamma", [1, 1], FP32, kind="ExternalInput")
    id_ext = nc.dram_tensor("ident", [128, 128], BF16, kind="ExternalInput")
    out_ext = nc.dram_tensor("out", [BPC, C, N], FP32, kind="ExternalOutput")

    with tile.TileContext(nc) as tc:
        with ExitStack() as ctx:
            if reps == 0:
                _noop_body(ctx, tc, nc, g_ext, out_ext)
            else:
                _body(ctx, tc, nc, x_ext, g_ext, id_ext, out_ext, reps)

    nc.compile()
    return nc


def _noop_body(ctx, tc, nc, g_ext, out_ext):
    pool = ctx.enter_context(tc.tile_pool(name="np", bufs=1))
    t = pool.tile([1, 1], FP32, name="t")
    nc.sync.dma_start(t[:], g_ext[:, :])
    nc.gpsimd.dma_start(out_ext[0, 0:1, 0:1], t[:])


def _body(ctx, tc, nc, x_ext, g_ext, id_ext, out_ext, reps=1):
    consts = ctx.enter_context(tc.tile_pool(name="consts", bufs=1))
    xin_pool = ctx.enter_context(tc.tile_pool(name="xin", bufs=4))
    vn_pool = ctx.enter_context(tc.tile_pool(name="vn", bufs=2 * CT))
    vt_pool = ctx.enter_context(tc.tile_pool(name="vt", bufs=NK))
    tx_pool = ctx.enter_context(tc.tile_pool(name="tx", bufs=CT))
    at_pool = ctx.enter_context(tc.tile_pool(name="at", bufs=CT))
    st_pool = ctx.enter_context(tc.tile_pool(name="st", bufs=2 * CT))
    out_pool = ctx.enter_context(tc.tile_pool(name="osb", bufs=4))

    ps_tr = ctx.enter_context(tc.tile_pool(name="ps_tr", bufs=2, space="PSUM"))
    ps_e = ctx.enter_context(tc.tile_pool(name="ps_e", bufs=2, space="PSUM"))
    ps_u = ctx.enter_context(tc.tile_pool(name="ps_u", bufs=2, space="PSUM"))

    ident = consts.tile([128, 128], BF16, name="ident")
    nc.sync.dma_start(ident[:], id_ext[:, :])

    gam = consts.tile([1, 1], FP32, name="gam")
    nc.sync.dma_start(gam[:], g_ext[:, :])
    gbc = consts.tile([128, 1], FP32, name="gbc")
    nc.gpsimd.partition_broadcast(gbc[:], gam[:], channels=128)

    for _rep in range(reps):
     for b in range(BPC):
        # ---- load + convert to bf16 (natural layout: c on partitions) ----
        vn = []
        for ct in range(CT):
            xin = xin_pool.tile([128, N], FP32, name="xin", tag="xin")
            nc.sync.dma_start(xin[:], x_ext[b, ct * 128:(ct + 1) * 128, :])
            v = vn_pool.tile([128, N], BF16, name="vn", tag="vn")
            nc.scalar.copy(v[:], xin[:])
            vn.append(v)

        # ---- transpose V -> Vt via identity matmuls (out = V_blk.T @ I) ----
        vt = []
        for nb in range(NK):
            ps = ps_tr.tile([128, 512], FP32, name="ps_tr", tag="ps_tr")
            for ct in range(CT):
                nc.tensor.matmul(
                    ps[:, ct * 128:(ct + 1) * 128],
                    lhsT=vn[ct][:, nb * 128:(nb + 1) * 128],
                    rhs=ident[:],
                    start=True,
                    stop=True,
                )
            t = vt_pool.tile([128, 512], BF16, name="vt", tag="vt")
            if nb % 2 == 0:
                nc.scalar.copy(t[:], ps[:])
            else:
                nc.vector.tensor_copy(t[:], ps[:])
            vt.append(t)

        # ---- E = V @ V.T  (per c-tile row of E), then exp(min - E) ----
        tx = []
        rsg = []
        for ct in range(CT):
            pse = ps_e.tile([128, 512], FP32, name="ps_e", tag="ps_e")
            for k in range(NK):
                nc.tensor.matmul(
                    pse[:],
                    lhsT=vt[k][:, ct * 128:(ct + 1) * 128],
                    rhs=vt[k][:],
                    start=(k == 0),
                    stop=(k == NK - 1),
                )
            mmin = st_pool.tile([128, 1], FP32, name="mmin", tag="mmin")
            nc.vector.tensor_reduce(
                out=mmin[:], in_=pse[:],
                axis=mybir.AxisListType.X, op=mybir.AluOpType.min,
            )
            t = tx_pool.tile([128, 512], BF16, name="tx", tag="tx")
            ssum = st_pool.tile([128, 1], FP32, name="ssum", tag="ssum")
            # t = exp(min_row(E) - E), ssum = rowsum(t)
            nc.scalar.activation(
                t[:], pse[:], mybir.ActivationFunctionType.Exp,
                bias=mmin[:], scale=-1.0, accum_out=ssum[:],
            )
            rs = st_pool.tile([128, 1], FP32, name="rs", tag="rs")
            nc.vector.reciprocal(rs[:], ssum[:])
            rg = st_pool.tile([128, 1], FP32, name="rg", tag="rg")
            nc.vector.tensor_mul(rg[:], rs[:], gbc[:])   # gamma / S_c
            tx.append(t)
            rsg.append(rg)

        # ---- A^T (unnormalized) via identity matmuls ----
        at = []
        for dj in range(CT):
            ps = ps_tr.tile([128, 512], FP32, name="ps_at", tag="ps_tr")
            for ct in range(CT):
                nc.tensor.matmul(
                    ps[:, ct * 128:(ct + 1) * 128],
                    lhsT=tx[ct][:, dj * 128:(dj + 1) * 128],
                    rhs=ident[:],
                    start=True,
                    stop=True,
                )
            a = at_pool.tile([128, 512], BF16, name="at", tag="at")
            nc.vector.tensor_copy(a[:], ps[:])
            at.append(a)

        # ---- U = T @ V ; out = (gamma/S_c) * U + x ----
        for ct in range(CT):
            for nb in range(NB):
                psu = ps_u.tile([128, 512], FP32, name="ps_u", tag="ps_u")
                for dj in range(CT):
                    nc.tensor.matmul(
                        psu[:],
                        lhsT=at[dj][:, ct * 128:(ct + 1) * 128],
                        rhs=vn[dj][:, nb * 512:(nb + 1) * 512],
                        start=(dj == 0),
                        stop=(dj == CT - 1),
                    )
                o = out_pool.tile([128, 512], FP32, name="osb", tag="osb")
                nc.vector.scalar_tensor_tensor(
                    out=o[:],
                    in0=psu[:],
                    scalar=rsg[ct][:],
                    in1=vn[ct][:, nb * 512:(nb + 1) * 512],
                    op0=mybir.AluOpType.mult,
                    op1=mybir.AluOpType.add,
                )
                nc.gpsimd.dma_start(
                    out_ext[b, ct * 128:(ct + 1) * 128, nb * 512:(nb + 1) * 512],
                    o[:],
                )


_NC_CACHE = {}


def _get_nc(reps=1):
    if reps not in _NC_CACHE:
        _NC_CACHE[reps] = _build_kernel(reps)
    return _NC_CACHE[reps]


def kernel(x: np.ndarray, gamma: np.ndarray) -> np.ndarray:
    assert x.shape == (B, C, HH, WW), x.shape
    nc = _get_nc()

    xr = np.ascontiguousarray(x, dtype=np.float32).reshape(B, C, N)
    g2 = np.asarray(gamma, dtype=np.float32).reshape(1, 1)
    ident = np.eye(128, dtype=ml_dtypes.bfloat16)

    in_maps = []
    for i in range(NCORES):
        in_maps.append({
            "x": xr[i * BPC:(i + 1) * BPC],
            "gamma": g2,
            "ident": ident,
        })

    res = run_bass_kernel_spmd(nc, in_maps, core_ids=list(range(NCORES)))
    outs = [res.results[i]["out"] for i in range(NCORES)]
    full = np.concatenate(outs, axis=0).reshape(B, C, HH, WW)
    return full.astype(np.float32)
